# revision 15
# baseline (speedup 1.0000x reference)
"""GATv2 2-layer GNN + classifier on 8 Trainium2 NeuronCores (Bass/Tile).

Sharding: nodes (and their incident edges, grouped by destination) are
sharded across the 8 cores; weights are replicated; per layer the source
projections xl are AllGathered so every core can dma_gather the rows for
its edges' sources.

Per dst-block of 128 nodes (edges sorted by dst, padded to equal counts
across cores so the SPMD program is identical):
  - dma_gather xl_full[src_e] -> X_g  [128 edges/partition-chunk, 1024]
  - PE: psum_m = Sd^T.T @ xr_local  (expand xr[dst] per edge)
        psum_m += I.T @ X_g         (m = xl[src] + xr[dst], all on PE)
  - ACT: lrelu = Prelu(psum_m, alpha=0.2)
  - DVE: e[:,h] = reduce_add(lrelu * att_bcast) per head
  - ACT: p = Exp(e);  Sep_h = Se * p[:,h] (per-partition scale)
  - PE: psum_out[:, h*256:] += Sep_h.T @ X_g[:, h*256:]   (scatter)
        psum_den[:, h]      += Sep_h.T @ ones             (softmax denom)
  - after block: out = psum_out * recip(psum_den+1e-16) + bias; LN; ELU
"""
import os
import sys

sys.path.insert(0, "/opt/trn_rl_repo")

import numpy as np
from contextlib import ExitStack

from concourse import bass, tile, mybir
from concourse.bacc import Bacc
from concourse.bass_utils import run_bass_kernel_spmd

f32 = mybir.dt.float32
i16 = mybir.dt.int16
AF = mybir.ActivationFunctionType
ALU = mybir.AluOpType

N_NODES = 10000
N_EDGES = 160000
IN_CH = 1030
HID = 256
HEADS = 4
HC = HID * HEADS  # 1024
OUT_CH = 49
NEG = 0.2
EPS = 1e-5
NCORES = 8
SHARD = N_NODES // NCORES  # 1250
NBLK = (SHARD + 127) // 128  # 10 blocks/core (9x128 + 98)

# const tile column layout (all [128, x] f32, rows replicated or identity)
_CW_ID = 0           # identity [128,128]
_CW_ATT1 = 128       # att1 bcast [128,1024]
_CW_ATT2 = 1152
_CW_BL1 = 2176       # c1_bl bcast
_CW_BR1 = 3200
_CW_BL2 = 4224
_CW_BR2 = 5248
_CW_C1B = 6272       # c1_bias
_CW_C2B = 7296
_CW_LN1W = 8320
_CW_LN1B = 9344
_CW_LN2W = 10368
_CW_LN2B = 11392
_CW_CB1 = 12416      # cls_b1 [128,256]
_CW_CB2 = 12672      # cls_b2 [128,49]
_CW_ONES = 12721     # ones [128,1]
_CW_EPS = 12722      # eps [128,1]
_CW_IOTA = 12723     # iota col [128,1]: partition index
_CW_IOTAR = 12724    # iota rows [128,128]: every row = 0..127
CONSTW = 12852


def _build_edge_tables(edge_index):
    """Per-core edge tables. Returns (E_pad[b] shared, per-core dicts)."""
    src = np.concatenate([edge_index[0], np.arange(N_NODES, dtype=np.int64)])
    dst = np.concatenate([edge_index[1], np.arange(N_NODES, dtype=np.int64)])
    order = np.argsort(dst, kind="stable")
    src, dst = src[order], dst[order]

    # per (core, block): edge slices
    counts = np.zeros((NCORES, NBLK), dtype=np.int64)
    segs = {}
    # boundaries of dst blocks globally: block index g = dst // 128 within core
    core_of = dst // SHARD
    dloc = dst - core_of * SHARD
    blk_of = dloc // 128
    for k in range(NCORES):
        m = core_of == k
        sk, dk = src[m], dloc[m]
        bk = blk_of[m]
        for b in range(NBLK):
            mb = bk == b
            segs[(k, b)] = (sk[mb], dk[mb] - b * 128)
            counts[k, b] = mb.sum()
    E_pad = [int(-(-counts[:, b].max() // 128) * 128) for b in range(NBLK)]

    cores = []
    for k in range(NCORES):
        srcs, dls = [], []
        for b in range(NBLK):
            s, d = segs[(k, b)]
            pad = E_pad[b] - len(s)
            srcs.append(np.concatenate([s, np.zeros(pad, dtype=np.int64)]))
            dls.append(np.concatenate([d, np.full(pad, -1, dtype=np.int64)]))
        s_all = np.concatenate(srcs)
        d_all = np.concatenate(dls)
        ecp = len(s_all)
        # wrapped int16 idxs: idx i -> [i%16 (replicated x8), i//16]
        s_m = _agmap(s_all)
        idx_w = np.tile(s_m.astype(np.int16).reshape(-1, 16).T, (8, 1)).copy()
        # Sd[d, e] = 1 if dst_local(e)==d ; Se[p, c*128+d] likewise for edge c*128+p
        Sd = np.zeros((128, ecp), dtype=np.float32)
        valid = d_all >= 0
        Sd[d_all[valid], np.nonzero(valid)[0]] = 1.0
        Se = np.zeros((128, ecp), dtype=np.float32)
        e_ids = np.nonzero(valid)[0]
        dv = d_all[valid]
        Se[e_ids % 128, (e_ids // 128) * 128 + dv] = 1.0
        cores.append({"idx_w": idx_w, "Sd": Sd, "Se": Se})
    return E_pad, cores


HALF_ROWS0 = 5 * 128          # blocks 0-4 rows per core
HALF_ROWS1 = SHARD - HALF_ROWS0  # blocks 5-9 rows per core


def _agmap(node_ids):
    """global node id -> row in the half-gathered xl_full layout."""
    k = node_ids // SHARD
    i = node_ids - k * SHARD
    first = i < HALF_ROWS0
    return np.where(first, k * HALF_ROWS0 + i,
                    NCORES * HALF_ROWS0 + k * HALF_ROWS1 + (i - HALF_ROWS0))


def _consts_np(inp):
    c = np.zeros((128, CONSTW), dtype=np.float32)
    c[:, _CW_ID:_CW_ID + 128] = np.eye(128, dtype=np.float32)
    def bcast(col, v):
        c[:, col:col + len(v)] = np.asarray(v, dtype=np.float32)[None, :]
    bcast(_CW_ATT1, inp["c1_att"].reshape(-1))
    bcast(_CW_ATT2, inp["c2_att"].reshape(-1))
    bcast(_CW_BL1, inp["c1_bl"]); bcast(_CW_BR1, inp["c1_br"])
    bcast(_CW_BL2, inp["c2_bl"]); bcast(_CW_BR2, inp["c2_br"])
    bcast(_CW_C1B, inp["c1_bias"]); bcast(_CW_C2B, inp["c2_bias"])
    bcast(_CW_LN1W, inp["ln1_w"]); bcast(_CW_LN1B, inp["ln1_b"])
    bcast(_CW_LN2W, inp["ln2_w"]); bcast(_CW_LN2B, inp["ln2_b"])
    bcast(_CW_CB1, inp["cls_b1"]); bcast(_CW_CB2, inp["cls_b2"])
    c[:, _CW_ONES] = 1.0
    c[:, _CW_EPS] = EPS
    c[:, _CW_IOTA] = np.arange(128)
    c[:, _CW_IOTAR:_CW_IOTAR + 128] = np.arange(128)[None, :]
    return c


def _rows(b):
    return min(128, SHARD - b * 128)


def _proj_phase(nc, tc, ctx, src_dram, w_l, w_r, bl_col, br_col, cst, out_ag,
                out_xr, in_ch, tag, src_is_T=False):
    """xl = src @ wl + bl -> out_ag ; xr = src @ wr + br -> out_xr.

    src_is_T: src_dram is [in_ch, SHARD] (host-pretransposed) -> no PE
    transposes needed; lhsT tiles DMA'd directly.
    """
    kt = [(i * 128, min(128, in_ch - i * 128)) for i in range(-(-in_ch // 128))]
    with tc.tile_pool(name=f"pw{tag}", bufs=1) as wpool, \
         tc.tile_pool(name=f"px{tag}", bufs=3) as xpool, \
         tc.tile_pool(name=f"pt{tag}", bufs=3) as tpool, \
         tc.tile_pool(name=f"po{tag}", bufs=2) as opool, \
         tc.tile_pool(name=f"qt{tag}", bufs=2, space="PSUM") as qt, \
         tc.tile_pool(name=f"qa{tag}", bufs=1, space="PSUM") as qa:
        wl_t, wr_t = [], []
        for (k0, kw) in kt:
            tl = wpool.tile([128, HC], f32, tag=f"wl{tag}{k0}")
            nc.sync.dma_start(tl[:kw, :], w_l[k0:k0 + kw, :])
            wl_t.append(tl)
            tr = wpool.tile([128, HC], f32, tag=f"wr{tag}{k0}")
            nc.sync.dma_start(tr[:kw, :], w_r[k0:k0 + kw, :])
            wr_t.append(tr)
        strips = []
        if src_is_T:
            for (k0, kw) in kt:
                st_t = wpool.tile([128, SHARD], f32, tag=f"xs{k0}")
                nc.sync.dma_start(st_t[:kw, :], src_dram[k0:k0 + kw, :])
                strips.append(st_t)
        for b in range(NBLK):
            rows = _rows(b)
            if not src_is_T:
                x_t = xpool.tile([128, in_ch], f32, tag="xblk")
                nc.sync.dma_start(x_t[:rows, :],
                                  src_dram[b * 128:b * 128 + rows, :])
            ps_l = qa.tile([128, HC], f32, tag="psl")
            ps_r = qa.tile([128, HC], f32, tag="psr")
            for ki, (k0, kw) in enumerate(kt):
                if src_is_T:
                    xT = strips[ki][:, b * 128:b * 128 + rows]
                else:
                    pt = qt.tile([128, 128], f32, tag="ptr")
                    nc.tensor.transpose(pt[:kw, :rows], x_t[:rows, k0:k0 + kw],
                                        cst[:rows, _CW_ID:_CW_ID + rows])
                    xT = tpool.tile([128, 128], f32, tag="xT")
                    nc.scalar.copy(xT[:kw, :rows], pt[:kw, :rows])
                st, sp = ki == 0, ki == len(kt) - 1
                lhs = xT[:kw, :rows] if not src_is_T else xT[:kw, :]
                for n0 in (0, 512):
                    nc.tensor.matmul(ps_l[:rows, n0:n0 + 512], lhs,
                                     wl_t[ki][:kw, n0:n0 + 512], start=st, stop=sp)
                    nc.tensor.matmul(ps_r[:rows, n0:n0 + 512], lhs,
                                     wr_t[ki][:kw, n0:n0 + 512], start=st, stop=sp)
            xl_s = opool.tile([128, HC], f32, tag="xls")
            nc.vector.tensor_add(xl_s[:rows, :], ps_l[:rows, :],
                                 cst[:rows, bl_col:bl_col + HC])
            xr_s = opool.tile([128, HC], f32, tag="xrs")
            nc.vector.tensor_add(xr_s[:rows, :], ps_r[:rows, :],
                                 cst[:rows, br_col:br_col + HC])
            ag_a, ag_b = out_ag
            if b * 128 < HALF_ROWS0:
                nc.sync.dma_start(ag_a[b * 128:b * 128 + rows, :], xl_s[:rows, :])
            else:
                r0 = b * 128 - HALF_ROWS0
                nc.sync.dma_start(ag_b[r0:r0 + rows, :], xl_s[:rows, :])
            nc.sync.dma_start(out_xr[b * 128:b * 128 + rows, :], xr_s[:rows, :])


def _ln_elu(nc, pool, cst, h_t, rows, w_col, b_col, tag):
    """In-place-ish LayerNorm + ELU on h_t[:rows, :HC]. Returns result tile."""
    stat = pool.tile([128, 8], f32, tag=f"st{tag}")
    scr = pool.tile([128, HC], f32, tag=f"sc{tag}")
    # mean & mean-square via ACT accumulate
    nc.scalar.activation(scr[:rows, :], h_t[:rows, :], AF.Copy,
                         accum_out=stat[:rows, 0:1])
    nc.scalar.activation(scr[:rows, :], h_t[:rows, :], AF.Square,
                         accum_out=stat[:rows, 1:2])
    mu = stat[:rows, 2:3]
    nc.vector.tensor_scalar_mul(mu, stat[:rows, 0:1], 1.0 / HC)
    msq = stat[:rows, 3:4]
    nc.vector.tensor_scalar_mul(msq, stat[:rows, 1:2], 1.0 / HC)
    mu2 = stat[:rows, 4:5]
    nc.vector.tensor_mul(mu2, mu, mu)
    var = stat[:rows, 5:6]
    nc.vector.tensor_sub(var, msq, mu2)
    sd = stat[:rows, 6:7]
    nc.scalar.activation(sd, var, AF.Sqrt, bias=cst[:rows, _CW_EPS:_CW_EPS + 1], scale=1.0)
    rstd = stat[:rows, 7:8]
    nc.vector.reciprocal(rstd, sd)
    nmu = stat[:rows, 4:5]  # reuse: -mu*rstd
    nc.vector.tensor_mul(nmu, mu, rstd)
    nc.vector.tensor_scalar_mul(nmu, nmu, -1.0)
    xn = pool.tile([128, HC], f32, tag=f"xn{tag}")
    nc.scalar.activation(xn[:rows, :], h_t[:rows, :], AF.Identity,
                         bias=nmu, scale=rstd)
    nc.vector.tensor_mul(xn[:rows, :], xn[:rows, :], cst[:rows, w_col:w_col + HC])
    nc.vector.tensor_add(xn[:rows, :], xn[:rows, :], cst[:rows, b_col:b_col + HC])
    # ELU: relu(x) + min(exp(x),1)-1
    ex = pool.tile([128, HC], f32, tag=f"ex{tag}")
    nc.scalar.activation(ex[:rows, :], xn[:rows, :], AF.Exp)
    nc.vector.tensor_scalar(ex[:rows, :], ex[:rows, :], 1.0, -1.0,
                            ALU.min, ALU.add)
    rl = pool.tile([128, HC], f32, tag=f"rl{tag}")
    nc.scalar.activation(rl[:rows, :], xn[:rows, :], AF.Relu)
    nc.vector.tensor_add(ex[:rows, :], ex[:rows, :], rl[:rows, :])
    return ex


def _edge_phase(nc, tc, ctx, E_pad, e_off, xl_full, xr_dram, idx_dram, sd_dram,
                se_dram, cst, att_col, cb_col, lnw_col, lnb_col, out_dram, tag):
    """One GAT conv layer's edge stage + LN + ELU. Writes out_dram [SHARD,HC]."""
    SLOT = 512
    with tc.tile_pool(name=f"eg{tag}", bufs=4) as gpool, \
         tc.tile_pool(name=f"es{tag}", bufs=2) as spool, \
         tc.tile_pool(name=f"ex{tag}", bufs=2) as xpool, \
         tc.tile_pool(name=f"ew{tag}", bufs=3) as wpool, \
         tc.tile_pool(name=f"ei{tag}", bufs=1) as ipool, \
         tc.tile_pool(name=f"eo{tag}", bufs=2) as opool, \
         tc.tile_pool(name=f"el{tag}", bufs=1) as lnpool, \
         tc.tile_pool(name=f"qm{tag}", bufs=3, space="PSUM") as qm, \
         tc.tile_pool(name=f"qo{tag}", bufs=2, space="PSUM") as qo, \
         tc.tile_pool(name=f"qd{tag}", bufs=1, space="PSUM") as qd:
        ecp = sum(E_pad)
        idx_t = ipool.tile([128, ecp // 16], i16, tag="idx")
        nc.sync.dma_start(idx_t[:], idx_dram[:])
        for b in range(NBLK):
            rows = _rows(b)
            e0 = e_off[b]
            eb = E_pad[b]
            xr_t = xpool.tile([128, HC], f32, tag="xr")
            nc.gpsimd.memset(xr_t[:], 0.0)
            nc.sync.dma_start(xr_t[:rows, :], xr_dram[b * 128:b * 128 + rows, :])
            ps_out = qo.tile([128, HC], f32, tag="pso")
            ps_den = qd.tile([128, 4], f32, tag="psd")
            nslot = -(-eb // SLOT)
            ci = 0
            for s in range(nslot):
                s0 = e0 + s * SLOT
                es = min(SLOT, eb - s * SLOT)
                g_t = gpool.tile([128, SLOT // 128, HC], f32, tag="gX")
                nc.gpsimd.dma_gather(
                    out_ap=g_t[:, :es // 128, :], in_ap=xl_full[:],
                    idxs_ap=idx_t[:, s0 // 16:(s0 + es) // 16],
                    num_idxs=es, num_idxs_reg=es, elem_size=HC)
                sd_t = spool.tile([128, SLOT], f32, tag="sd")
                nc.sync.dma_start(sd_t[:, :es], sd_dram[:, s0:s0 + es])
                se_t = spool.tile([128, SLOT], f32, tag="se")
                nc.sync.dma_start(se_t[:, :es], se_dram[:, s0:s0 + es])
                for c in range(es // 128):
                    first, last = ci == 0, ci == (eb // 128) - 1
                    lr = wpool.tile([128, HC], f32, tag="lr")
                    for n0 in (0, 512):
                        ps_m = qm.tile([128, 512], f32, tag="psm")
                        nc.tensor.matmul(ps_m[:],
                                         sd_t[:, c * 128:(c + 1) * 128],
                                         xr_t[:, n0:n0 + 512],
                                         start=True, stop=False)
                        nc.tensor.matmul(ps_m[:],
                                         cst[:, _CW_ID:_CW_ID + 128],
                                         g_t[:, c, n0:n0 + 512],
                                         start=False, stop=True)
                        nc.scalar.activation(lr[:, n0:n0 + 512], ps_m[:],
                                             AF.Prelu, alpha=NEG)
                    prod = wpool.tile([128, 4, HID], f32, tag="prod")
                    nc.vector.tensor_mul(
                        prod[:].rearrange("p h c -> p (h c)"), lr[:],
                        cst[:, att_col:att_col + HC])
                    escore = wpool.tile([128, 4], f32, tag="esc")
                    nc.vector.tensor_reduce(out=escore[:], in_=prod[:],
                                            axis=mybir.AxisListType.X,
                                            op=ALU.add)
                    p_t = wpool.tile([128, 4], f32, tag="pt")
                    nc.scalar.activation(p_t[:], escore[:], AF.Exp)
                    sep = wpool.tile([128, 4, 128], f32, tag="sep")
                    # one start=True per PSUM bank per block: start clears
                    # has_written for the WHOLE bank; unset bits -> overwrite
                    for h in range(HEADS):
                        nc.scalar.activation(sep[:, h, :],
                                             se_t[:, c * 128:(c + 1) * 128],
                                             AF.Copy, scale=p_t[:, h:h + 1])
                        nc.tensor.matmul(ps_out[:, h * HID:(h + 1) * HID],
                                         sep[:, h, :],
                                         g_t[:, c, h * HID:(h + 1) * HID],
                                         start=first and h % 2 == 0,
                                         stop=last and h % 2 == 1)
                    nc.tensor.matmul(ps_den[:, 0:4],
                                     se_t[:, c * 128:(c + 1) * 128], p_t[:],
                                     start=first, stop=last)
                    ci += 1
            den = opool.tile([128, 8], f32, tag="den")
            nc.vector.tensor_scalar_add(den[:rows, 0:4], ps_den[:rows, 0:4],
                                        1e-16)
            nc.vector.reciprocal(den[:rows, 4:8], den[:rows, 0:4])
            h_t = opool.tile([128, HC], f32, tag="hb")
            for h in range(HEADS):
                nc.scalar.activation(h_t[:rows, h * HID:(h + 1) * HID],
                                     ps_out[:rows, h * HID:(h + 1) * HID],
                                     AF.Copy, scale=den[:rows, 4 + h:5 + h])
            nc.vector.tensor_add(h_t[:rows, :], h_t[:rows, :],
                                 cst[:rows, cb_col:cb_col + HC])
            res = _ln_elu(nc, lnpool, cst, h_t, rows, lnw_col, lnb_col, tag)
            nc.sync.dma_start(out_dram[b * 128:b * 128 + rows, :],
                              res[:rows, :])


def _cls_phase(nc, tc, ctx, h2_dram, w1_dram, w2_dram, cst, out_ext):
    with tc.tile_pool(name="cw", bufs=1) as wpool, \
         tc.tile_pool(name="cx", bufs=3) as xpool, \
         tc.tile_pool(name="ct", bufs=3) as tpool, \
         tc.tile_pool(name="co", bufs=2) as opool, \
         tc.tile_pool(name="cq", bufs=2, space="PSUM") as qt, \
         tc.tile_pool(name="cqa", bufs=2, space="PSUM") as qa:
        w1_t = []
        for k in range(8):
            t = wpool.tile([128, HID], f32, tag=f"cw1{k}")
            nc.sync.dma_start(t[:], w1_dram[k * 128:(k + 1) * 128, :])
            w1_t.append(t)
        w2_t = []
        for k in range(2):
            t = wpool.tile([128, OUT_CH], f32, tag=f"cw2{k}")
            nc.sync.dma_start(t[:], w2_dram[k * 128:(k + 1) * 128, :])
            w2_t.append(t)
        for b in range(NBLK):
            rows = _rows(b)
            h_t = xpool.tile([128, HC], f32, tag="h2")
            nc.sync.dma_start(h_t[:rows, :], h2_dram[b * 128:b * 128 + rows, :])
            ps1 = qa.tile([128, HID], f32, tag="ps1")
            for k in range(8):
                pt = qt.tile([128, 128], f32, tag="ctr")
                nc.tensor.transpose(pt[:, :rows], h_t[:rows, k * 128:(k + 1) * 128],
                                    cst[:rows, _CW_ID:_CW_ID + rows])
                hT = tpool.tile([128, 128], f32, tag="hT")
                nc.scalar.copy(hT[:, :rows], pt[:, :rows])
                nc.tensor.matmul(ps1[:rows, :], hT[:, :rows], w1_t[k][:],
                                 start=k == 0, stop=k == 7)
            a1 = opool.tile([128, HID], f32, tag="a1")
            nc.vector.tensor_add(a1[:rows, :], ps1[:rows, :],
                                 cst[:rows, _CW_CB1:_CW_CB1 + HID])
            ex = opool.tile([128, HID], f32, tag="cex")
            nc.scalar.activation(ex[:rows, :], a1[:rows, :], AF.Exp)
            nc.vector.tensor_scalar(ex[:rows, :], ex[:rows, :], 1.0, -1.0,
                                    ALU.min, ALU.add)
            rl = opool.tile([128, HID], f32, tag="crl")
            nc.scalar.activation(rl[:rows, :], a1[:rows, :], AF.Relu)
            nc.vector.tensor_add(ex[:rows, :], ex[:rows, :], rl[:rows, :])
            ps2 = qa.tile([128, OUT_CH], f32, tag="ps2")
            for k in range(2):
                pt = qt.tile([128, 128], f32, tag="ctr")
                nc.tensor.transpose(pt[:, :rows], ex[:rows, k * 128:(k + 1) * 128],
                                    cst[:rows, _CW_ID:_CW_ID + rows])
                eT = tpool.tile([128, 128], f32, tag="eT")
                nc.scalar.copy(eT[:, :rows], pt[:, :rows])
                nc.tensor.matmul(ps2[:rows, :], eT[:, :rows], w2_t[k][:],
                                 start=k == 0, stop=k == 1)
            o_t = opool.tile([128, OUT_CH], f32, tag="ot")
            nc.vector.tensor_add(o_t[:rows, :], ps2[:rows, :],
                                 cst[:rows, _CW_CB2:_CW_CB2 + OUT_CH])
            nc.gpsimd.dma_start(out_ext[b * 128:b * 128 + rows, :], o_t[:rows, :])


def build_program(E_pad):
    e_off = [0]
    for b in range(NBLK):
        e_off.append(e_off[-1] + E_pad[b])
    ecp = e_off[-1]

    nc = Bacc()
    xT_shard = nc.declare_dram_parameter("xT_shard", [IN_CH, SHARD], f32, isOutput=False)
    idx_d = nc.declare_dram_parameter("idx_w", [128, ecp // 16], i16, isOutput=False)
    sd_d = nc.declare_dram_parameter("Sd", [128, ecp], f32, isOutput=False)
    se_d = nc.declare_dram_parameter("Se", [128, ecp], f32, isOutput=False)
    cst_d = nc.declare_dram_parameter("consts", [128, CONSTW], f32, isOutput=False)
    w1l = nc.declare_dram_parameter("w1l", [IN_CH, HC], f32, isOutput=False)
    w1r = nc.declare_dram_parameter("w1r", [IN_CH, HC], f32, isOutput=False)
    w2l = nc.declare_dram_parameter("w2l", [HC, HC], f32, isOutput=False)
    w2r = nc.declare_dram_parameter("w2r", [HC, HC], f32, isOutput=False)
    cw1 = nc.declare_dram_parameter("cls_w1", [HC, HID], f32, isOutput=False)
    cw2 = nc.declare_dram_parameter("cls_w2", [HID, OUT_CH], f32, isOutput=False)
    out_ext = nc.declare_dram_parameter("out", [SHARD, OUT_CH], f32, isOutput=True)

    ag_in1a = nc.dram_tensor("ag_in1a", [HALF_ROWS0, HC], f32)
    ag_in1b = nc.dram_tensor("ag_in1b", [HALF_ROWS1, HC], f32)
    xl1_full = nc.dram_tensor("xl1_full", [N_NODES, HC], f32, addr_space="Shared")
    xr1_d = nc.dram_tensor("xr1", [SHARD, HC], f32)
    h1_d = nc.dram_tensor("h1", [SHARD, HC], f32)
    ag_in2a = nc.dram_tensor("ag_in2a", [HALF_ROWS0, HC], f32)
    ag_in2b = nc.dram_tensor("ag_in2b", [HALF_ROWS1, HC], f32)
    xl2_full = nc.dram_tensor("xl2_full", [N_NODES, HC], f32, addr_space="Shared")
    xr2_d = nc.dram_tensor("xr2", [SHARD, HC], f32)
    h2_d = nc.dram_tensor("h2", [SHARD, HC], f32)

    rg = [list(range(NCORES))]
    with tile.TileContext(nc) as tc, ExitStack() as ctx:
        cpool = ctx.enter_context(tc.tile_pool(name="consts", bufs=1))
        cst = cpool.tile([128, CONSTW], f32, tag="cst")
        nc.gpsimd.dma_start(cst[:], cst_d[:])
        cstv = cst[:]

        _proj_phase(nc, tc, ctx, xT_shard, w1l, w1r, _CW_BL1, _CW_BR1, cstv,
                    (ag_in1a, ag_in1b), xr1_d, IN_CH, "1", src_is_T=True)
        h0 = HALF_ROWS0
        nc.gpsimd.collective_compute("AllGather", ALU.bypass, replica_groups=rg,
                                     ins=[ag_in1a[:]], outs=[xl1_full[0:NCORES * h0]])
        nc.gpsimd.collective_compute("AllGather", ALU.bypass, replica_groups=rg,
                                     ins=[ag_in1b[:]], outs=[xl1_full[NCORES * h0:]])
        _edge_phase(nc, tc, ctx, E_pad, e_off, xl1_full, xr1_d, idx_d, sd_d,
                    se_d, cstv, _CW_ATT1, _CW_C1B, _CW_LN1W, _CW_LN1B, h1_d, "1")
        _proj_phase(nc, tc, ctx, h1_d, w2l, w2r, _CW_BL2, _CW_BR2, cstv,
                    (ag_in2a, ag_in2b), xr2_d, HC, "2")
        nc.gpsimd.collective_compute("AllGather", ALU.bypass, replica_groups=rg,
                                     ins=[ag_in2a[:]], outs=[xl2_full[0:NCORES * h0]])
        nc.gpsimd.collective_compute("AllGather", ALU.bypass, replica_groups=rg,
                                     ins=[ag_in2b[:]], outs=[xl2_full[NCORES * h0:]])
        _edge_phase(nc, tc, ctx, E_pad, e_off, xl2_full, xr2_d, idx_d, sd_d,
                    se_d, cstv, _CW_ATT2, _CW_C2B, _CW_LN2W, _CW_LN2B, h2_d, "2")
        _cls_phase(nc, tc, ctx, h2_d, cw1, cw2, cstv, out_ext)
    nc.finalize()
    return nc


_CACHE = {}
LAST_RESULTS = None


def kernel(**inputs):
    global LAST_RESULTS
    inp = {k: np.asarray(v) for k, v in inputs.items()}
    edge_index = inp["edge_index"].astype(np.int64)
    key = hash(edge_index.tobytes())
    if key not in _CACHE:
        E_pad, cores = _build_edge_tables(edge_index)
        nc = build_program(E_pad)
        _CACHE[key] = (nc, cores)
    nc, cores = _CACHE[key]

    consts = _consts_np(inp)
    x = np.ascontiguousarray(inp["x"], dtype=np.float32)
    shared = {
        "consts": consts,
        "w1l": np.ascontiguousarray(inp["c1_wl"], dtype=np.float32),
        "w1r": np.ascontiguousarray(inp["c1_wr"], dtype=np.float32),
        "w2l": np.ascontiguousarray(inp["c2_wl"], dtype=np.float32),
        "w2r": np.ascontiguousarray(inp["c2_wr"], dtype=np.float32),
        "cls_w1": np.ascontiguousarray(inp["cls_w1"], dtype=np.float32),
        "cls_w2": np.ascontiguousarray(inp["cls_w2"], dtype=np.float32),
    }
    in_maps = []
    for k in range(NCORES):
        m = dict(shared)
        m["xT_shard"] = np.ascontiguousarray(x[k * SHARD:(k + 1) * SHARD].T)
        m["idx_w"] = cores[k]["idx_w"]
        m["Sd"] = cores[k]["Sd"]
        m["Se"] = cores[k]["Se"]
        in_maps.append(m)

    trace = bool(int(os.environ.get("KERNEL_TRACE", "0")))
    res = run_bass_kernel_spmd(nc, in_maps, list(range(NCORES)), trace=trace)
    LAST_RESULTS = res
    out = np.concatenate([res.results[k]["out"] for k in range(NCORES)], axis=0)
    return out
